# revision 4
# baseline (speedup 1.0000x reference)
"""Trainium2 Bass kernel for BinaryLinear: out = x @ sign(W).T  (v3)

Shapes (hardcoded): x [32768, 2048] f32, weight [2048, 2048] f32,
out [32768, 2048] f32.

Data-parallel over 8 NeuronCores: 4096 tokens/core, weight replicated.
Host-side prep is pure data movement: x is fed pre-tiled as
xt[tt, i_p, ic, t_l] = x[128*tt+t_l, 128*ic+i_p]; the weight is fed as
the HIGH 2 BYTES of each f32 of W.T (a byte-slice view == bf16
truncation, which preserves the sign of every representable w
exactly), so the weight prologue moves 8 MB instead of 16 MB.

On-device per core:
  - 48 dummy N=64 matmuls fill the otherwise-idle first ~2.5 us so the
    PE HAM clock gate is already released when the real stream begins.
  - prologue: 16 weight chunks [128, 2048] u16 stream in order on the
    gpsimd queue (~25 us; x tiles own the sync queue); sign() runs on
    VectorE for 9 chunks (mult 1e32 -> clip to [-1,1], exact for this
    data, ~0.6 us each, no act-table load) and ScalarE (Sign
    activation, ~1.9 us each) for 7, producing swT[ic] [128, 2048]
    bf16 resident in SBUF. The first chunks are split so the first
    real matmul issues ~2 us in; two token tiles (8 PSUM banks) ride
    the sign frontier, keeping the PE ~97% busy through the prologue.
  - steady state per 128-token tile: one 1 MB x load (sync queue), DVE
    cast f32->bf16, then ic-outer/oc-inner matmuls: the stationary xT
    chunk is loaded once per ic and reused for 4 N=512 matmuls into 4
    PSUM banks (8 banks total -> two tiles in flight, so PSUM drains
    never gate the PE; a stationary switch every matmul measured ~98ns
    extra on HW, amortized 4x here). Drains on VectorE (ScalarE
    activation-copies measured ~1us slower each on HW); stores are
    per-oc [128,512] chunks on gpsimd (+sync for the last tile).
"""

import os
import sys

if "/opt/trn_rl_repo" not in sys.path:
    sys.path.insert(0, "/opt/trn_rl_repo")

import numpy as np

T, I, O = 32768, 2048, 2048
NCORES = 8
TL = T // NCORES  # tokens per core

_NC = None


def _build(reps=1):
    import concourse.bacc as bacc
    import concourse.mybir as mybir
    from concourse import tile
    from contextlib import ExitStack

    f32 = mybir.dt.float32
    bf16 = mybir.dt.bfloat16
    u16 = mybir.dt.uint16

    IC = I // 128  # contraction chunks
    NT = TL // 128  # token tiles per core
    OCW = 512  # matmul moving free dim
    NOC = O // OCW

    DVE_SIGN_ICS = {3, 5, 7, 9, 11, 13}

    nc = bacc.Bacc("TRN2", target_bir_lowering=False, debug=False, num_devices=NCORES)
    xt = nc.dram_tensor("xt", [NT, 128, IC, 128], f32, kind="ExternalInput")
    wt = nc.dram_tensor("wt", [I, O], u16, kind="ExternalInput")
    out = nc.dram_tensor("out", [TL, O], f32, kind="ExternalOutput")

    with tile.TileContext(nc) as tc, ExitStack() as ctx:
        swt_pool = ctx.enter_context(tc.tile_pool(name="swt", bufs=1))
        swT = [swt_pool.tile([128, O], bf16, name=f"swT{ic}") for ic in range(IC)]

        wprep = ctx.enter_context(tc.tile_pool(name="wprep", bufs=1))
        w16 = [
            wprep.tile([128, O], u16, tag="w16", name=f"w16_{ic}", bufs=4)
            for ic in range(IC)
        ]

        # --- weight prologue ---
        def load_w(ic, eng):
            if ic == 0:
                for q in range(4):
                    eng.dma_start(
                        w16[0][:, OCW * q : OCW * (q + 1)],
                        wt[0:128, OCW * q : OCW * (q + 1)],
                    )
            elif ic in (1, 2):
                for h in range(2):
                    eng.dma_start(
                        w16[ic][:, 2 * OCW * h : 2 * OCW * (h + 1)],
                        wt[128 * ic : 128 * (ic + 1), 2 * OCW * h : 2 * OCW * (h + 1)],
                    )
            else:
                eng.dma_start(w16[ic][:], wt[128 * ic : 128 * (ic + 1), :])

        def dve_sign(dst, src_u16):
            # sign via saturating clip: w * 1e32 clipped to [-1, 1]
            # (exact +-1 for every nonzero bf16 w here; 0 stays 0)
            nc.vector.tensor_scalar(
                dst, src_u16.bitcast(bf16), 1e32, 1.0,
                mybir.AluOpType.mult, mybir.AluOpType.min,
            )
            nc.vector.tensor_scalar_max(dst, dst, -1.0)

        def sign_w(ic):
            w_bf = w16[ic][:].bitcast(bf16)
            if ic == 0:
                for q in range(4):
                    dve_sign(
                        swT[0][:, OCW * q : OCW * (q + 1)],
                        w16[0][:, OCW * q : OCW * (q + 1)],
                    )
            elif ic in (1, 2):
                for h in range(2):
                    dve_sign(
                        swT[ic][:, 2 * OCW * h : 2 * OCW * (h + 1)],
                        w16[ic][:, 2 * OCW * h : 2 * OCW * (h + 1)],
                    )
            elif ic in DVE_SIGN_ICS:
                dve_sign(swT[ic][:], w16[ic][:])
            else:
                nc.scalar.activation(
                    swT[ic][:], w_bf, mybir.ActivationFunctionType.Sign
                )

        xpool = ctx.enter_context(tc.tile_pool(name="xpool", bufs=3))
        xbpool = ctx.enter_context(tc.tile_pool(name="xbpool", bufs=3))
        opool = ctx.enter_context(tc.tile_pool(name="opool", bufs=3))
        psum_mm = ctx.enter_context(tc.tile_pool(name="psum_mm", bufs=8, space="PSUM"))

        # PE warmup: dummy matmuls fill the otherwise-idle first ~3us so
        # the HAM clock gate is released before the real stream begins.
        # They write a PSUM slot that later real accs overwrite (start=True).
        wup_pool = ctx.enter_context(tc.tile_pool(name="wup", bufs=1))
        wup = wup_pool.tile([128, 128], bf16, name="wup_sb")
        nc.vector.memset(wup[:], 0.0)
        wacc = psum_mm.tile([128, OCW], f32, tag="acc", name="wup_acc")
        for _ in range(48):
            nc.tensor.matmul(wacc[:, 0:64], wup[:], wup[:, 0:64], start=True, stop=True)

        # prologue: W chunks stream on gpsimd (sole queue, in order); x on
        # sync; signs on ACT (+ DVE's share); first chunks quartered
        load_w(0, nc.gpsimd)
        x_f32_pre = {}
        xT_pre = {}
        for tt in (0, 1):
            x_f32_pre[tt] = xpool.tile(
                [128, IC, 128], f32, tag="x_f32", name=f"x_f32_0_{tt}"
            )
            xT_pre[tt] = xbpool.tile(
                [128, IC, 128], bf16, tag="xT", name=f"xT_0_{tt}"
            )
        for q in range(4):
            nc.sync.dma_start(
                x_f32_pre[0][:, 4 * q : 4 * (q + 1), :], xt[0, :, 4 * q : 4 * (q + 1), :]
            )
            nc.vector.tensor_copy(
                xT_pre[0][:, 4 * q : 4 * (q + 1), :],
                x_f32_pre[0][:, 4 * q : 4 * (q + 1), :],
            )
        for ic in range(1, IC):
            load_w(ic, nc.gpsimd)
        nc.sync.dma_start(x_f32_pre[1][:], xt[1])
        nc.vector.tensor_copy(xT_pre[1][:], x_f32_pre[1][:])
        for ic in range(IC):
            sign_w(ic)

        for rep in range(reps):
            first = rep == 0
            for tt in range(NT):
                if first and tt in (0, 1):
                    x_f32, xT = x_f32_pre[tt], xT_pre[tt]
                else:
                    x_f32 = xpool.tile(
                        [128, IC, 128], f32, tag="x_f32", name=f"x_f32_{rep}_{tt}"
                    )
                    xT = xbpool.tile([128, IC, 128], bf16, tag="xT", name=f"xT_{rep}_{tt}")
                    nc.sync.dma_start(x_f32[:], xt[tt])
                    nc.vector.tensor_copy(xT[:], x_f32[:])

                accs = [
                    psum_mm.tile([128, OCW], f32, tag="acc", name=f"acc_{rep}_{tt}_{oc}")
                    for oc in range(NOC)
                ]
                for ic in range(IC):
                    for oc in range(NOC):
                        nc.tensor.matmul(
                            accs[oc][:],
                            xT[:, ic, :],
                            swT[ic][:, OCW * oc : OCW * (oc + 1)],
                            start=(ic == 0),
                            stop=(ic == IC - 1),
                        )
                o_sb = opool.tile([128, O], f32, tag="o_sb", name=f"o_sb_{rep}_{tt}")
                last = tt == NT - 1 and rep == reps - 1
                for oc in range(NOC):
                    nc.vector.tensor_copy(
                        o_sb[:, OCW * oc : OCW * (oc + 1)], accs[oc][:]
                    )
                    seng = nc.sync if (last and oc % 2 == 1) else nc.gpsimd
                    seng.dma_start(
                        out[128 * tt : 128 * (tt + 1), OCW * oc : OCW * (oc + 1)],
                        o_sb[:, OCW * oc : OCW * (oc + 1)],
                    )

    nc.compile()
    return nc


def _get_nc():
    global _NC
    if _NC is None:
        _NC = _build(reps=1)
    return _NC


def _in_maps(x, w):
    x = np.asarray(x, dtype=np.float32)
    w = np.asarray(w, dtype=np.float32)
    assert x.shape == (T, I) and w.shape == (O, I)
    xt = np.ascontiguousarray(
        x.reshape(T // 128, 128, I // 128, 128).transpose(0, 3, 2, 1)
    )
    # high 2 bytes of each f32 of W.T == bf16 truncation; preserves sign
    wtT = np.ascontiguousarray(w.T)
    wt16 = np.ascontiguousarray(wtT.view(np.uint16).reshape(I, O, 2)[:, :, 1])
    ntl = TL // 128
    return [{"xt": xt[c * ntl : (c + 1) * ntl], "wt": wt16} for c in range(NCORES)]


def kernel(**inputs):
    from concourse.bass_utils import run_bass_kernel_spmd

    nc = _get_nc()
    res = run_bass_kernel_spmd(
        nc, _in_maps(inputs["x"], inputs["weight"]), core_ids=list(range(NCORES))
    )
    return np.concatenate([r["out"] for r in res.results], axis=0)
